# revision 1
# baseline (speedup 1.0000x reference)
"""Trainium2 Bass kernel for nn_Attention_85710367359290 (sparse branch-routed attention).

Semantics (validated vs reference, rel err ~2e-6 in numpy):
  q = rope(a @ Wq) per branch (NB=4), k = rope(x @ Wk), v = a @ Wv per branch
  att[b,n,t,s] = q.k/sqrt(C);  m = max_n att;  p = exp(m) (no max-sub, |att|<~8)
  routing: combined_n = p * (att_n >= m) on causal positions
  y = sum_n combined_n @ v_n;  Z = sum_s p;  out = (y/Z) @ Wo

Two-phase SPMD over 8 cores (no collectives; host reshuffles between phases):
  Phase A: projections + rope, fully distributed — core i owns a 512-row T-slice
           of batch i//4, computes qrT/krT slices (f32r, split-rope layout) and
           v slice (bf16).
  Phase B: attention — core (b,j) owns t-rows [256j,..) u [2048-256(j+1),..)
           (causally balanced). Uniform program: causal masks shipped as data,
           s-loops padded (lo-block 8 s-chunks, hi-block 16).

Phase-B layouts (partition dim first):
  qrT [c'=16x128, t=512], krT [c'=4x128, s=2048] (f32r, split-permuted+rope'd)
  v [s, c'] bf16 streamed per s-chunk; attT[s=128, t=256]/branch in PSUM
  pv: yT[c,t] += v_n^T-chunks x combined_n; o = (yT/Z)^T @ Wo
"""

import os
import numpy as np
import ml_dtypes

import concourse.bass as bass
import concourse.mybir as mybir
import concourse.tile as tile
from concourse import bacc
from concourse.bass_utils import run_bass_kernel_spmd

F32 = mybir.dt.float32
F32R = mybir.dt.float32r
BF16 = mybir.dt.bfloat16
ALU = mybir.AluOpType
ACTF = mybir.ActivationFunctionType

B, T, C, NB = 2, 2048, 512, 4
TB = 256
SC = 128
LO_TRIPS, HI_TRIPS = 8, 16
N_CORES = 8

QK_F32R = True
QKD = F32R if QK_F32R else F32
VD = BF16
NPVD = ml_dtypes.bfloat16

_cache = {}
REPEAT = int(os.environ.get("KREPEAT", "1"))
KLOOP = int(os.environ.get("KLOOP", "0"))  # device-side repeat loop (timing)


class _NullCtx:
    def __enter__(self):
        return 0

    def __exit__(self, *a):
        return False


def _loop(tc):
    return tc.For_i(0, KLOOP, 1) if KLOOP > 1 else _NullCtx()


def build_phase_a():
    if "a" in _cache:
        return _cache["a"]
    nc = bacc.Bacc("TRN2", target_bir_lowering=False, debug=False)

    def din(name, shape, dt):
        return nc.dram_tensor(name, shape, dt, kind="ExternalInput").ap()

    aT = din("aT", [C, 512], QKD)      # a[b].T columns of this core's T-slice
    aTb = din("aTb", [C, 512], VD)     # same, bf16 (for v)
    xT = din("xT", [C, 512], QKD)
    Wq = din("Wq", [C, NB * C], QKD)   # split-permuted
    Wk = din("Wk", [C, C], QKD)        # split-permuted, pre-scaled 1/sqrt(C)
    Wv = din("Wv", [C, NB * C], VD)
    cosA = din("cosA", [C // 2, 512], F32)
    sinA = din("sinA", [C // 2, 512], F32)
    qrA = nc.dram_tensor("qrA", [NB * C, 512], QKD, kind="ExternalOutput").ap()
    krA = nc.dram_tensor("krA", [C, 512], QKD, kind="ExternalOutput").ap()
    vA = nc.dram_tensor("vA", [512, NB * C], VD, kind="ExternalOutput").ap()

    with tile.TileContext(nc) as tc:
        with (
            tc.tile_pool(name="pa", bufs=1) as pa,
            tc.tile_pool(name="pat", bufs=4) as pat,
            tc.tile_pool(name="pap", bufs=8, space="PSUM") as pps,
        ):
            aTt = [pa.tile([128, 512], QKD, tag=f"aT{i}", name=f"aT{i}") for i in range(4)]
            aTbt = [pa.tile([128, 512], VD, tag=f"aTb{i}", name=f"aTb{i}") for i in range(4)]
            xTt = [pa.tile([128, 512], QKD, tag=f"xT{i}", name=f"xT{i}") for i in range(4)]
            WqT = [pa.tile([128, NB * C], QKD, tag=f"Wq{i}", name=f"Wq{i}") for i in range(4)]
            WkT = [pa.tile([128, C], QKD, tag=f"Wk{i}", name=f"Wk{i}") for i in range(4)]
            WvT = [pa.tile([128, NB * C], VD, tag=f"Wv{i}", name=f"Wv{i}") for i in range(4)]
            cst = [pa.tile([128, 512], F32, tag=f"cs{i}", name=f"cs{i}") for i in range(2)]
            snt = [pa.tile([128, 512], F32, tag=f"sn{i}", name=f"sn{i}") for i in range(2)]
            for i in range(4):
                nc.sync.dma_start(out=WkT[i], in_=Wk[i * 128:(i + 1) * 128, :])
                nc.sync.dma_start(out=xTt[i], in_=xT[i * 128:(i + 1) * 128, :])
            for i in range(2):
                nc.sync.dma_start(out=cst[i], in_=cosA[i * 128:(i + 1) * 128, :])
                nc.sync.dma_start(out=snt[i], in_=sinA[i * 128:(i + 1) * 128, :])
            for i in range(4):
                nc.sync.dma_start(out=aTt[i], in_=aT[i * 128:(i + 1) * 128, :])
                nc.sync.dma_start(out=WqT[i], in_=Wq[i * 128:(i + 1) * 128, :])
            for i in range(4):
                nc.sync.dma_start(out=aTbt[i], in_=aTb[i * 128:(i + 1) * 128, :])
                nc.sync.dma_start(out=WvT[i], in_=Wv[i * 128:(i + 1) * 128, :])

            # ---- kT proj + rope: krA[c', s-slice] ----
            kpre = [pa.tile([128, 512], F32, tag=f"kpre{i}", name=f"kpre{i}") for i in range(4)]
            _lc = _loop(tc)
            _lc.__enter__()
            for _r in range(REPEAT):
             for m in range(4):
                ps = pps.tile([128, 512], F32, tag="pps", name="pps")
                for Kc in range(4):
                    nc.tensor.matmul(ps, WkT[Kc][:, m * 128:(m + 1) * 128], xTt[Kc],
                                     start=(Kc == 0), stop=(Kc == 3))
                nc.scalar.copy(out=kpre[m], in_=ps)
            for _r in range(REPEAT):
             for h in range(2):
                t1 = pat.tile([128, 512], F32, tag="t1", name="t1")
                t2 = pat.tile([128, 512], F32, tag="t2", name="t2")
                kr = pat.tile([128, 512], QKD, tag="kr", name="kr")
                nc.vector.tensor_mul(t1, kpre[h], cst[h])
                nc.vector.tensor_mul(t2, kpre[2 + h], snt[h])
                nc.vector.tensor_sub(kr, t1, t2)
                nc.sync.dma_start(out=krA[h * 128:(h + 1) * 128, :], in_=kr)
                t3 = pat.tile([128, 512], F32, tag="t3", name="t3")
                t4 = pat.tile([128, 512], F32, tag="t4", name="t4")
                kr2 = pat.tile([128, 512], QKD, tag="kr2", name="kr2")
                nc.vector.tensor_mul(t3, kpre[h], snt[h])
                nc.vector.tensor_mul(t4, kpre[2 + h], cst[h])
                nc.vector.tensor_add(kr2, t3, t4)
                nc.sync.dma_start(out=krA[(2 + h) * 128:(3 + h) * 128, :], in_=kr2)

            # ---- q proj + rope: qrA[c', t-slice] ----
            qpre = [pa.tile([128, 512], F32, tag=f"qpre{i}", name=f"qpre{i}") for i in range(4)]
            for _r in range(REPEAT):
             for n in range(NB):
                for m in range(4):
                    ps = pps.tile([128, 512], F32, tag="pps", name="pps")
                    for Kc in range(4):
                        nc.tensor.matmul(
                            ps, WqT[Kc][:, (4 * n + m) * 128:(4 * n + m + 1) * 128],
                            aTt[Kc], start=(Kc == 0), stop=(Kc == 3))
                    nc.scalar.copy(out=qpre[m], in_=ps)
                for h in range(2):
                    t1 = pat.tile([128, 512], F32, tag="qt1", name="qt1")
                    t2 = pat.tile([128, 512], F32, tag="qt2", name="qt2")
                    qr = pat.tile([128, 512], QKD, tag="qkr", name="qr")
                    nc.vector.tensor_mul(t1, qpre[h], cst[h])
                    nc.vector.tensor_mul(t2, qpre[2 + h], snt[h])
                    nc.vector.tensor_sub(qr, t1, t2)
                    nc.sync.dma_start(
                        out=qrA[(4 * n + h) * 128:(4 * n + h + 1) * 128, :], in_=qr)
                    t3 = pat.tile([128, 512], F32, tag="qt3", name="qt3")
                    t4 = pat.tile([128, 512], F32, tag="qt4", name="qt4")
                    qr2 = pat.tile([128, 512], QKD, tag="qkr2", name="qr2")
                    nc.vector.tensor_mul(t3, qpre[h], snt[h])
                    nc.vector.tensor_mul(t4, qpre[2 + h], cst[h])
                    nc.vector.tensor_add(qr2, t3, t4)
                    nc.sync.dma_start(
                        out=qrA[(4 * n + 2 + h) * 128:(4 * n + 3 + h) * 128, :], in_=qr2)
            # ---- v proj: vA[s-slice, c'] ----
            for _r in range(REPEAT):
             for sc in range(4):
                for nb in range(4):
                    ps = pps.tile([128, 512], F32, tag="pps", name="pps")
                    for Kc in range(4):
                        nc.tensor.matmul(ps, aTbt[Kc][:, sc * 128:(sc + 1) * 128],
                                         WvT[Kc][:, nb * 512:(nb + 1) * 512],
                                         start=(Kc == 0), stop=(Kc == 3))
                    vs = pat.tile([128, 512], VD, tag="vs", name="vs")
                    nc.scalar.copy(out=vs, in_=ps)
                    nc.sync.dma_start(
                        out=vA[sc * 128:(sc + 1) * 128, nb * 512:(nb + 1) * 512], in_=vs)

            _lc.__exit__(None, None, None)
    nc.compile()
    _cache["a"] = nc
    return nc


def build_phase_b():
    if "b" in _cache:
        return _cache["b"]
    nc = bacc.Bacc("TRN2", target_bir_lowering=False, debug=False)

    def din(name, shape, dt):
        return nc.dram_tensor(name, shape, dt, kind="ExternalInput").ap()

    qp = din("qp", [8 * 128, 1024], QKD)   # (Kc,bp) tiles: [brE-lo|brO-lo|brE-hi|brO-hi]
    krB = din("krB", [C, T], QKD)
    vB = din("vB", [T, NB * C], VD)
    WoD = din("Wo", [C, C], VD)
    mlo = din("mlo", [LO_TRIPS // 2, SC, 512], BF16)
    mhi = din("mhi", [HI_TRIPS // 2, SC, 512], BF16)
    out = nc.dram_tensor("o", [512, C], F32, kind="ExternalOutput").ap()

    with tile.TileContext(nc) as tc:
        with (
            tc.tile_pool(name="persist", bufs=1) as pp,
            tc.tile_pool(name="attw", bufs=3) as aw,
            tc.tile_pool(name="atts", bufs=6) as asts,
            tc.tile_pool(name="attp", bufs=1, space="PSUM") as app,
            tc.tile_pool(name="accp", bufs=1, space="PSUM") as acc,
            tc.tile_pool(name="opsp", bufs=1, space="PSUM") as opsp,
        ):
            qpT = [pp.tile([128, 1024], QKD, tag=f"qp{i}", name=f"qp{i}") for i in range(8)]
            krT = [pp.tile([128, T], QKD, tag=f"krT{i}", name=f"krT{i}") for i in range(4)]
            WoT = [pp.tile([128, C], VD, tag=f"Wo{i}", name=f"Wo{i}") for i in range(4)]
            ones = pp.tile([128, 1], BF16, tag="ones", name="ones")
            nc.vector.memset(ones, 1.0)
            # split loads: lo-halves / early s-columns first so si=0 starts early
            for i in range(8):
                nc.sync.dma_start(out=qpT[i][:, :512], in_=qp[i * 128:(i + 1) * 128, :512])
            for cb in range(4):
                for i in range(4):
                    nc.sync.dma_start(
                        out=krT[i][:, cb * 512:(cb + 1) * 512],
                        in_=krB[i * 128:(i + 1) * 128, cb * 512:(cb + 1) * 512])
            for i in range(8):
                nc.sync.dma_start(out=qpT[i][:, 512:], in_=qp[i * 128:(i + 1) * 128, 512:])
            for i in range(4):
                nc.sync.dma_start(out=WoT[i], in_=WoD[i * 128:(i + 1) * 128, :])

            _lc = _loop(tc)
            _lc.__enter__()
            for _r in range(REPEAT):
             for tb, (trips, mskd) in enumerate([(LO_TRIPS, mlo), (HI_TRIPS, mhi)]):
                toff = tb * 512
                npair = trips // 2
                yT = [acc.tile([128, 512], F32, tag=f"yT{i}", name=f"yT{i}") for i in range(2)]
                Zp = acc.tile([128, 8], F32, tag="Zp", name="Zp")
                for pr in range(npair):
                    vsi = []
                    for sp in range(2):
                        si = 2 * pr + sp
                        vt = asts.tile([128, NB * C], VD, tag="vsi", name="vsi")
                        nc.sync.dma_start(out=vt, in_=vB[si * 128:(si + 1) * 128, :])
                        vsi.append(vt)
                    msk = asts.tile([SC, 512], BF16, tag="msk", name="msk")
                    nc.sync.dma_start(out=msk, in_=mskd[pr, :, :])
                    att = [[app.tile([128, 512], F32, tag=f"att{bp}{sp}", name=f"att{bp}{sp}")
                            for sp in range(2)] for bp in range(2)]
                    for sp in range(2):
                        si = 2 * pr + sp
                        for Kc in range(4):
                            for bp in range(2):
                                nc.tensor.matmul(
                                    att[bp][sp],
                                    krT[Kc][:, si * 128:(si + 1) * 128],
                                    qpT[Kc * 2 + bp][:, toff:toff + 512],
                                    start=(Kc == 0), stop=(Kc == 3),
                                )
                    # e[bp] cols: [sp0-brE | sp0-brO | sp1-brE | sp1-brO]
                    e = [aw.tile([128, 1024], F32, tag=f"e{i}", name=f"e{i}") for i in range(2)]
                    for bp in range(2):
                        for sp in range(2):
                            nc.scalar.activation(
                                out=e[bp][:, sp * 512:(sp + 1) * 512],
                                in_=att[bp][sp], func=ACTF.Exp)

                    def pview(t1024, par):
                        return t1024.rearrange("p (sp par c) -> p sp par c",
                                               sp=2, par=2)[:, :, par, :]

                    def v2(t512):
                        return t512.rearrange("p (sp c) -> p sp c", sp=2)

                    pm = [aw.tile([128, 512], F32, tag=f"pm{i}", name=f"pm{i}")
                          for i in range(2)]
                    pmax = aw.tile([128, 512], F32, tag="pmax", name="pmax")
                    for bp in range(2):
                        nc.vector.tensor_max(v2(pm[bp]), pview(e[bp], 0), pview(e[bp], 1))
                    nc.vector.tensor_max(pmax, pm[0], pm[1])
                    p_m = aw.tile([128, 512], BF16, tag="p_m", name="p_m")
                    nc.vector.tensor_mul(p_m, pmax, msk)
                    # mb/cmb are par-major [par, sp, c] so writes are contiguous
                    # (bf16 step-1 => DVE 2x mode for the mul pass)
                    mb = [aw.tile([128, 1024], BF16, tag=f"mb{i}", name=f"mb{i}")
                          for i in range(2)]
                    cmb = [aw.tile([128, 1024], VD, tag=f"cmb{i}", name=f"cmb{i}")
                           for i in range(2)]
                    for bp in range(2):
                        for par in range(2):
                            psl = slice(par * 512, (par + 1) * 512)
                            nc.vector.tensor_tensor(
                                out=v2(mb[bp][:, psl]), in0=pview(e[bp], par),
                                in1=v2(pmax), op=ALU.is_ge)
                            nc.vector.tensor_mul(
                                cmb[bp][:, psl], mb[bp][:, psl], p_m)
                    for sp in range(2):
                        for tc_ in range(2):
                            nc.tensor.matmul(
                                Zp[:, tb * 2 + tc_:tb * 2 + tc_ + 1],
                                p_m[:, sp * 256 + tc_ * 128:sp * 256 + (tc_ + 1) * 128],
                                ones,
                                start=(pr == 0 and sp == 0 and tc_ == 0),
                                stop=(pr == npair - 1 and sp == 1 and tc_ == 1))
                    for sp in range(2):
                        for br in range(4):
                            bp, par = br // 2, br % 2
                            rsl = slice(par * 512 + sp * 256, par * 512 + sp * 256 + 256)
                            for Mc in range(4):
                                nc.tensor.matmul(
                                    yT[Mc // 2][:, (Mc % 2) * 256:(Mc % 2) * 256 + 256],
                                    vsi[sp][:, br * 512 + Mc * 128:br * 512 + (Mc + 1) * 128],
                                    cmb[bp][:, rsl],
                                    start=(pr == 0 and sp == 0 and br == 0 and Mc % 2 == 0),
                                    stop=(pr == npair - 1 and sp == 1 and br == 3 and Mc % 2 == 1))
                # epilogue
                yb = [aw.tile([128, 512], VD, tag=f"yb{i}", name=f"yb{i}") for i in range(2)]
                for i in range(2):
                    nc.scalar.copy(out=yb[i], in_=yT[i])
                zr = aw.tile([128, 2], F32, tag="zr", name="zr")
                nc.vector.reciprocal(zr, Zp[:, tb * 2:tb * 2 + 2])
                for tc_ in range(2):
                    ops = opsp.tile([128, 512], F32, tag="ops", name="ops")
                    for Kc in range(4):
                        nc.tensor.matmul(
                            ops,
                            yb[Kc // 2][:, (Kc % 2) * 256 + tc_ * 128:(Kc % 2) * 256 + (tc_ + 1) * 128],
                            WoT[Kc], start=(Kc == 0), stop=(Kc == 3))
                    osb = aw.tile([128, 512], F32, tag="osb", name="osb")
                    nc.vector.tensor_scalar_mul(osb, ops, zr[:, tc_:tc_ + 1])
                    nc.sync.dma_start(
                        out=out[tb * 256 + tc_ * 128:tb * 256 + (tc_ + 1) * 128, :], in_=osb)
            _lc.__exit__(None, None, None)
    nc.compile()
    _cache["b"] = nc
    return nc


def _masks(j):
    lo, hi = 256 * j, T - 256 * (j + 1)
    m_lo = np.zeros((LO_TRIPS // 2, SC, 2, TB), np.float32)
    m_hi = np.zeros((HI_TRIPS // 2, SC, 2, TB), np.float32)
    tt = np.arange(TB)[None, :]
    ss = np.arange(SC)[:, None]
    for pr in range(LO_TRIPS // 2):
        for sp in range(2):
            m_lo[pr, :, sp, :] = (lo + tt) >= ((2 * pr + sp) * SC + ss)
    for pr in range(HI_TRIPS // 2):
        for sp in range(2):
            m_hi[pr, :, sp, :] = (hi + tt) >= ((2 * pr + sp) * SC + ss)
    return (m_lo.reshape(LO_TRIPS // 2, SC, 512).astype(ml_dtypes.bfloat16),
            m_hi.reshape(HI_TRIPS // 2, SC, 512).astype(ml_dtypes.bfloat16))


def kernel(a, x, Wq, Wk, Wv, Wo, cos, sin, _trace=False):
    a = np.asarray(a, np.float32)
    x = np.asarray(x, np.float32)
    Wq = np.asarray(Wq, np.float32)
    Wk = np.asarray(Wk, np.float32)
    Wv = np.asarray(Wv, np.float32)
    Wo = np.asarray(Wo, np.float32)
    cos = np.asarray(cos, np.float32)
    sin = np.asarray(sin, np.float32)

    split_idx = np.r_[0:C:2, 1:C:2]
    Wq_p = np.ascontiguousarray(Wq.reshape(C, NB, C)[:, :, split_idx].reshape(C, NB * C))
    Wk_p = np.ascontiguousarray(Wk[:, split_idx] * np.float32(1.0 / np.sqrt(C)))
    Wv_b = Wv.astype(NPVD)
    Wo_b = Wo.astype(NPVD)
    cosTf = np.ascontiguousarray(cos[:T].T)
    sinTf = np.ascontiguousarray(sin[:T].T)

    # ---- phase A ----
    nca = build_phase_a()
    in_a = []
    for core in range(N_CORES):
        b, s4 = divmod(core, 4)
        rows = slice(512 * s4, 512 * (s4 + 1))
        aTs = np.ascontiguousarray(a[b].T[:, rows])
        in_a.append({
            "aT": aTs,
            "aTb": aTs.astype(NPVD),
            "xT": np.ascontiguousarray(x[b].T[:, rows]),
            "Wq": Wq_p, "Wk": Wk_p, "Wv": Wv_b,
            "cosA": np.ascontiguousarray(cosTf[:, rows]),
            "sinA": np.ascontiguousarray(sinTf[:, rows]),
        })
    res_a = run_bass_kernel_spmd(nca, in_a, list(range(N_CORES)))

    # host reshuffle: full qr/kr/v per batch
    qr_full = [np.concatenate([res_a.results[b * 4 + s]["qrA"] for s in range(4)], axis=1)
               for b in range(B)]   # [2048, 2048]
    kr_full = [np.concatenate([res_a.results[b * 4 + s]["krA"] for s in range(4)], axis=1)
               for b in range(B)]   # [512, 2048]
    v_full = [np.concatenate([res_a.results[b * 4 + s]["vA"] for s in range(4)], axis=0)
              for b in range(B)]    # [2048, 2048] bf16

    # ---- phase B ----
    ncb = build_phase_b()
    in_b = []
    for core in range(N_CORES):
        b, j = divmod(core, 4)
        lo, hi = 256 * j, T - 256 * (j + 1)
        m_lo, m_hi = _masks(j)
        qpk = np.empty((8 * 128, 1024), np.float32)
        for Kc in range(4):
            for bp in range(2):
                r = Kc * 2 + bp
                for half, cs in ((0, slice(lo, lo + 256)), (1, slice(hi, hi + 256))):
                    for par in range(2):
                        br = 2 * bp + par
                        qpk[r * 128:(r + 1) * 128,
                            half * 512 + par * 256:half * 512 + par * 256 + 256] = \
                            qr_full[b][(4 * br + Kc) * 128:(4 * br + Kc + 1) * 128, cs]
        in_b.append({
            "qp": qpk,
            "krB": kr_full[b],
            "vB": v_full[b],
            "Wo": Wo_b,
            "mlo": m_lo, "mhi": m_hi,
        })
    res_b = run_bass_kernel_spmd(ncb, in_b, list(range(N_CORES)))

    outf = np.zeros((B, T, C), np.float32)
    for core in range(N_CORES):
        b, j = divmod(core, 4)
        lo, hi = 256 * j, T - 256 * (j + 1)
        o = res_b.results[core]["o"]
        outf[b, lo:lo + 256] = o[:256]
        outf[b, hi:hi + 256] = o[256:]
    if _trace:
        return outf, (res_a, res_b)
    return outf



# revision 5
# speedup vs baseline: 1.0287x; 1.0287x over previous
"""Trainium2 Bass kernel for nn_Attention_85710367359290 (sparse branch-routed attention).

Semantics (validated vs reference offline, rel err ~0.009):
  q = rope(a @ Wq) per branch (NB=4), k = rope(x @ Wk), v = a @ Wv per branch
  att[b,n,t,s] = q.k/sqrt(C);  amax = max_n exp(att);  p = amax (no max-sub, |att|<~8)
  routing: cmb_n = p * (exp(att_n) >= amax) on causal positions
  y = sum_n cmb_n @ v_n;  Z = sum_s p;  out = (y/Z) @ Wo

Two-phase SPMD over 8 cores (no collectives; host reshuffles between phases).
All tensor data fp16 (PSUM accum f32); offline numerics: rel err ~0.009.

Phase A: k-proj+rope and v-proj, core (b,s4) owns a 512-row s-slice of batch b.
Phase B: q-proj+rope + attention. Core (b,j) owns 4 t-blocks of 128 rows:
  tb = {15-j, 11-j, 7-j, 3-j} assigned to slots 0..3 with fixed per-slot
  s-chunk capacities (16,12,8,4) >= needs (16-j,12-j,8-j,4-j): a uniform
  40-unit program (si-major so each v s-chunk is loaded once: 16 loads).
  Padded units are neutralized by all-zero masks (data).

Unit (si, slot): att[s128, (n4,t128)] = kr_si^T q_slot (4 matmuls ap512 fp16);
  e = exp(att-4) fp16; amax = max_n e (2 STT); p_m = amax*msk; mb = e>=amax
  (bcast); cmb = mb*p_m (bcast); Z[:,slot] += colsum (PE ones-matmul);
  y[slot] += v_n^T cmb_n (16 matmuls ap128). Epilogue: o = (y/Z)^T @ Wo.
"""

import numpy as np

import concourse.bass as bass
import concourse.mybir as mybir
import concourse.tile as tile
from concourse import bacc
from concourse.bass_utils import run_bass_kernel_spmd

F32 = mybir.dt.float32
F16 = mybir.dt.float16
ALU = mybir.AluOpType
ACTF = mybir.ActivationFunctionType
NPF16 = np.float16

B, T, C, NB = 2, 2048, 512, 4
SC = 128          # s-chunk (PSUM partition dim)
BLK = 128         # t-block
NSI = T // SC     # 16
CAPS = (16, 12, 8, 4)
UNITS = [(si, sl) for si in range(NSI) for sl in range(4) if si < CAPS[sl]]
N_CORES = 8
EXP_BIAS = -4.0

_cache = {}


def _stt(eng, out, in0, in1, op):
    eng.scalar_tensor_tensor(out=out, in0=in0, scalar=1.0, in1=in1,
                             op0=ALU.mult, op1=op)


def build_phase_a():
    if "a" in _cache:
        return _cache["a"]
    nc = bacc.Bacc("TRN2", target_bir_lowering=False, debug=False)

    def din(name, shape, dt):
        return nc.dram_tensor(name, shape, dt, kind="ExternalInput").ap()

    xT = din("xT", [C, 512], F16)       # x[b].T cols of this core's s-slice
    aTv = din("aTv", [C, 512], F16)     # a[b].T same cols (for v)
    Wk = din("Wk", [C, C], F16)         # split-permuted, pre-scaled 1/sqrt(C)
    Wv = din("Wv", [C, NB * C], F16)
    cosA = din("cosA", [C // 2, 512], F16)
    sinA = din("sinA", [C // 2, 512], F16)
    krA = nc.dram_tensor("krA", [C, 512], F16, kind="ExternalOutput").ap()
    vA = nc.dram_tensor("vA", [512, NB * C], F16, kind="ExternalOutput").ap()

    with tile.TileContext(nc) as tc:
        with (
            tc.tile_pool(name="pa", bufs=1) as pa,
            tc.tile_pool(name="pat", bufs=4) as pat,
            tc.tile_pool(name="pap", bufs=6, space="PSUM") as pps,
        ):
            xTt = [pa.tile([128, 512], F16, tag=f"xT{i}", name=f"xT{i}") for i in range(4)]
            aTt = [pa.tile([128, 512], F16, tag=f"aT{i}", name=f"aT{i}") for i in range(4)]
            WkT = [pa.tile([128, C], F16, tag=f"Wk{i}", name=f"Wk{i}") for i in range(4)]
            WvT = [pa.tile([128, NB * C], F16, tag=f"Wv{i}", name=f"Wv{i}") for i in range(4)]
            cst = [pa.tile([128, 512], F16, tag=f"cs{i}", name=f"cs{i}") for i in range(2)]
            snt = [pa.tile([128, 512], F16, tag=f"sn{i}", name=f"sn{i}") for i in range(2)]
            for i in range(4):
                nc.sync.dma_start(out=WkT[i], in_=Wk[i * 128:(i + 1) * 128, :])
                nc.sync.dma_start(out=xTt[i], in_=xT[i * 128:(i + 1) * 128, :])
            for i in range(2):
                nc.sync.dma_start(out=cst[i], in_=cosA[i * 128:(i + 1) * 128, :])
                nc.sync.dma_start(out=snt[i], in_=sinA[i * 128:(i + 1) * 128, :])
            for i in range(4):
                nc.sync.dma_start(out=aTt[i], in_=aTv[i * 128:(i + 1) * 128, :])
                nc.sync.dma_start(out=WvT[i], in_=Wv[i * 128:(i + 1) * 128, :])

            # ---- k proj -> fp16 copy -> rope -> krA ----
            kp16 = [pa.tile([128, 512], F16, tag=f"kp{i}", name=f"kp{i}") for i in range(4)]
            for m in range(4):
                ps = pps.tile([128, 512], F32, tag="pps", name="pps")
                for Kc in range(4):
                    nc.tensor.matmul(ps, WkT[Kc][:, m * 128:(m + 1) * 128], xTt[Kc],
                                     start=(Kc == 0), stop=(Kc == 3))
                nc.scalar.copy(out=kp16[m], in_=ps)
            for h in range(2):
                t1 = pat.tile([128, 512], F16, tag="t1", name="t1")
                t2 = pat.tile([128, 512], F16, tag="t2", name="t2")
                kr = pat.tile([128, 512], F16, tag="kr", name="kr")
                _stt(nc.vector, t1, kp16[h], cst[h], ALU.mult)
                _stt(nc.vector, t2, kp16[2 + h], snt[h], ALU.mult)
                _stt(nc.vector, kr, t1, t2, ALU.subtract)
                nc.sync.dma_start(out=krA[h * 128:(h + 1) * 128, :], in_=kr)
                t3 = pat.tile([128, 512], F16, tag="t3", name="t3")
                t4 = pat.tile([128, 512], F16, tag="t4", name="t4")
                kr2 = pat.tile([128, 512], F16, tag="kr2", name="kr2")
                _stt(nc.vector, t3, kp16[h], snt[h], ALU.mult)
                _stt(nc.vector, t4, kp16[2 + h], cst[h], ALU.mult)
                _stt(nc.vector, kr2, t3, t4, ALU.add)
                nc.sync.dma_start(out=krA[(2 + h) * 128:(3 + h) * 128, :], in_=kr2)

            # ---- v proj: vA[s-slice, (n,c)] ----
            for sc in range(4):
                for nb in range(4):
                    ps = pps.tile([128, 512], F32, tag="pps", name="pps")
                    for Kc in range(4):
                        nc.tensor.matmul(ps, aTt[Kc][:, sc * 128:(sc + 1) * 128],
                                         WvT[Kc][:, nb * 512:(nb + 1) * 512],
                                         start=(Kc == 0), stop=(Kc == 3))
                    vs = pat.tile([128, 512], F16, tag="vs", name="vs")
                    nc.scalar.copy(out=vs, in_=ps)
                    nc.sync.dma_start(
                        out=vA[sc * 128:(sc + 1) * 128, nb * 512:(nb + 1) * 512], in_=vs)
    nc.compile()
    _cache["a"] = nc
    return nc


def build_phase_b():
    if "b" in _cache:
        return _cache["b"]
    nc = bacc.Bacc("TRN2", target_bir_lowering=False, debug=False)

    def din(name, shape, dt):
        return nc.dram_tensor(name, shape, dt, kind="ExternalInput").ap()

    aQ = din("aQ", [C, 512], F16)       # a[b].T cols = 4 t-blocks (slot order)
    Wq = din("Wq", [C, NB * C], F16)    # split-permuted
    cosB = din("cosB", [C // 2, 512], F16)
    sinB = din("sinB", [C // 2, 512], F16)
    krB = din("krB", [C, T], F16)
    vB = din("vB", [T, NB * C], F16)
    WoD = din("Wo", [C, C], F16)
    mskd = din("mskd", [len(UNITS), SC, BLK], F16)
    out = nc.dram_tensor("o", [512, C], F32, kind="ExternalOutput").ap()

    with tile.TileContext(nc) as tc:
        with (
            tc.tile_pool(name="persist", bufs=1) as pp,
            tc.tile_pool(name="qtmp", bufs=4) as qtp,
            tc.tile_pool(name="vstream", bufs=3) as vsp,
            tc.tile_pool(name="ew", bufs=3) as ew,
            tc.tile_pool(name="ep", bufs=2) as epi,
        ):
            aQt = [pp.tile([128, 512], F16, tag=f"aQ{i}", name=f"aQ{i}") for i in range(4)]
            WqT = [pp.tile([128, NB * C], F16, tag=f"Wq{i}", name=f"Wq{i}") for i in range(4)]
            cst = [pp.tile([128, 512], F16, tag=f"cs{i}", name=f"cs{i}") for i in range(2)]
            snt = [pp.tile([128, 512], F16, tag=f"sn{i}", name=f"sn{i}") for i in range(2)]
            krT = [pp.tile([128, T], F16, tag=f"krT{i}", name=f"krT{i}") for i in range(4)]
            # qrT[m]: partition = c' in chunk m; free = (n 4, t 512)
            qrT = [pp.tile([128, NB * 512], F16, tag=f"qr{i}", name=f"qr{i}") for i in range(4)]
            WoT = [pp.tile([128, C], F16, tag=f"Wo{i}", name=f"Wo{i}") for i in range(4)]
            ones = pp.tile([128, 1], F16, tag="ones", name="ones")
            ebias = pp.tile([128, 1], F32, tag="ebias", name="ebias")
            nc.vector.memset(ones, 1.0)
            nc.vector.memset(ebias, EXP_BIAS)
            for i in range(4):
                nc.sync.dma_start(out=aQt[i], in_=aQ[i * 128:(i + 1) * 128, :])
                nc.sync.dma_start(out=WqT[i], in_=Wq[i * 128:(i + 1) * 128, :])
            for i in range(2):
                nc.sync.dma_start(out=cst[i], in_=cosB[i * 128:(i + 1) * 128, :])
                nc.sync.dma_start(out=snt[i], in_=sinB[i * 128:(i + 1) * 128, :])
            # early s-chunks of kr first so the si loop can start ASAP
            for cb in range(4):
                for i in range(4):
                    nc.sync.dma_start(
                        out=krT[i][:, cb * 512:(cb + 1) * 512],
                        in_=krB[i * 128:(i + 1) * 128, cb * 512:(cb + 1) * 512])
            for i in range(4):
                nc.sync.dma_start(out=WoT[i], in_=WoD[i * 128:(i + 1) * 128, :])

            with tc.tile_pool(name="qpp", bufs=4, space="PSUM") as qpp:
                # ---- q proj + rope (all 4 blocks at once) ----
                for n in range(NB):
                    qp16 = [qtp.tile([128, 512], F16, tag=f"qp{m}", name=f"qp{m}")
                            for m in range(4)]
                    for m in range(4):
                        ps = qpp.tile([128, 512], F32, tag="qps", name="qps")
                        for Kc in range(4):
                            nc.tensor.matmul(
                                ps, WqT[Kc][:, (4 * n + m) * 128:(4 * n + m + 1) * 128],
                                aQt[Kc], start=(Kc == 0), stop=(Kc == 3))
                        nc.scalar.copy(out=qp16[m], in_=ps)
                    for h in range(2):
                        t1 = qtp.tile([128, 512], F16, tag="qt1", name="qt1")
                        t2 = qtp.tile([128, 512], F16, tag="qt2", name="qt2")
                        _stt(nc.vector, t1, qp16[h], cst[h], ALU.mult)
                        _stt(nc.vector, t2, qp16[2 + h], snt[h], ALU.mult)
                        _stt(nc.vector, qrT[h][:, n * 512:(n + 1) * 512], t1, t2,
                             ALU.subtract)
                        t3 = qtp.tile([128, 512], F16, tag="qt3", name="qt3")
                        t4 = qtp.tile([128, 512], F16, tag="qt4", name="qt4")
                        _stt(nc.vector, t3, qp16[h], snt[h], ALU.mult)
                        _stt(nc.vector, t4, qp16[2 + h], cst[h], ALU.mult)
                        _stt(nc.vector, qrT[2 + h][:, n * 512:(n + 1) * 512], t3, t4,
                             ALU.add)

            with (
                tc.tile_pool(name="attp", bufs=2, space="PSUM") as app,
                tc.tile_pool(name="accp", bufs=1, space="PSUM") as acc,
                tc.tile_pool(name="opsp", bufs=1, space="PSUM") as opsp,
            ):
                yT = [acc.tile([128, 512], F32, tag=f"yT{i}", name=f"yT{i}")
                      for i in range(4)]
                Zp = acc.tile([128, 4], F32, tag="Zp", name="Zp")
                qv = [qrT[Kc].rearrange("p (n t) -> p n t", n=NB) for Kc in range(4)]
                ui = 0
                for si in range(NSI):
                    vt = vsp.tile([128, NB * C], F16, tag="vsi", name="vsi")
                    nc.sync.dma_start(out=vt, in_=vB[si * 128:(si + 1) * 128, :])
                    for sl in range(4):
                        if si >= CAPS[sl]:
                            continue
                        start, stop = si == 0, si == CAPS[sl] - 1
                        msk = ew.tile([SC, BLK], F16, tag="msk", name="msk")
                        nc.sync.dma_start(out=msk, in_=mskd[ui, :, :])
                        att = app.tile([128, NB * BLK], F32, tag="att", name="att")
                        for Kc in range(4):
                            nc.tensor.matmul(
                                att.rearrange("p (n t) -> p n t", n=NB),
                                krT[Kc][:, si * 128:(si + 1) * 128],
                                qv[Kc][:, :, sl * 128:(sl + 1) * 128],
                                start=(Kc == 0), stop=(Kc == 3))
                        e = ew.tile([128, NB * BLK], F16, tag="e", name="e")
                        nc.scalar.activation(out=e, in_=att, func=ACTF.Exp,
                                             bias=ebias[:, 0:1])
                        m1 = ew.tile([128, 256], F16, tag="m1", name="m1")
                        amax = ew.tile([128, BLK], F16, tag="amax", name="amax")
                        p_m = ew.tile([128, BLK], F16, tag="p_m", name="p_m")
                        _stt(nc.vector, m1, e[:, 0:256], e[:, 256:512], ALU.max)
                        _stt(nc.vector, amax, m1[:, 0:128], m1[:, 128:256], ALU.max)
                        _stt(nc.vector, p_m, amax, msk, ALU.mult)
                        amax_b = amax.rearrange("p (a t) -> p a t", a=1).to_broadcast(
                            [128, NB, BLK])
                        pm_b = p_m.rearrange("p (a t) -> p a t", a=1).to_broadcast(
                            [128, NB, BLK])
                        mb = ew.tile([128, NB * BLK], F16, tag="mb", name="mb")
                        cmb = ew.tile([128, NB * BLK], F16, tag="cmb", name="cmb")
                        ev = e.rearrange("p (n t) -> p n t", n=NB)
                        _stt(nc.vector, mb.rearrange("p (n t) -> p n t", n=NB),
                             ev, amax_b, ALU.is_ge)
                        _stt(nc.vector, cmb.rearrange("p (n t) -> p n t", n=NB),
                             mb.rearrange("p (n t) -> p n t", n=NB), pm_b, ALU.mult)
                        # NB: matmul start=True zeroes the whole PSUM bank, so
                        # only the chronologically-first matmul per bank sets it
                        nc.tensor.matmul(Zp[:, sl:sl + 1], p_m, ones,
                                         start=(start and sl == 0), stop=stop)
                        for br in range(NB):
                            for Mc in range(4):
                                nc.tensor.matmul(
                                    yT[sl][:, Mc * 128:(Mc + 1) * 128],
                                    vt[:, br * 512 + Mc * 128:br * 512 + (Mc + 1) * 128],
                                    cmb[:, br * 128:(br + 1) * 128],
                                    start=(start and br == 0 and Mc == 0),
                                    stop=(stop and br == 3))
                        ui += 1

                # ---- epilogue per slot: o = (y/Z)^T @ Wo ----
                for sl in range(4):
                    yb = epi.tile([128, 512], F16, tag="yb", name="yb")
                    nc.scalar.copy(out=yb, in_=yT[sl])
                    zr = epi.tile([128, 1], F32, tag="zr", name="zr")
                    nc.vector.reciprocal(zr, Zp[:, sl:sl + 1])
                    ops = opsp.tile([128, 512], F32, tag="ops", name="ops")
                    for Mc in range(4):
                        nc.tensor.matmul(ops, yb[:, Mc * 128:(Mc + 1) * 128], WoT[Mc],
                                         start=(Mc == 0), stop=(Mc == 3))
                    osb = epi.tile([128, 512], F32, tag="osb", name="osb")
                    nc.vector.tensor_scalar_mul(osb, ops, zr[:, 0:1])
                    nc.sync.dma_start(out=out[sl * 128:(sl + 1) * 128, :], in_=osb)
    nc.compile()
    _cache["b"] = nc
    return nc


def _host_prep(Wq, Wk, Wv, Wo, cos, sin):
    split_idx = np.r_[0:C:2, 1:C:2]
    Wq_p = np.ascontiguousarray(
        Wq.reshape(C, NB, C)[:, :, split_idx].reshape(C, NB * C)).astype(NPF16)
    Wk_p = (Wk[:, split_idx] * np.float32(1.0 / np.sqrt(C))).astype(NPF16)
    Wv_h = Wv.astype(NPF16)
    Wo_h = Wo.astype(NPF16)
    cosT = np.ascontiguousarray(cos[:T].T).astype(NPF16)  # [C/2, T]
    sinT = np.ascontiguousarray(sin[:T].T).astype(NPF16)
    return Wq_p, Wk_p, Wv_h, Wo_h, cosT, sinT


def _core_blocks(j):
    return [15 - j, 11 - j, 7 - j, 3 - j]


def _masks(j):
    m = np.zeros((len(UNITS), SC, BLK), np.float32)
    tbs = _core_blocks(j)
    tt = np.arange(BLK)[None, :]
    ss = np.arange(SC)[:, None]
    for ui, (si, sl) in enumerate(UNITS):
        t0 = BLK * tbs[sl]
        m[ui] = (t0 + tt) >= (SC * si + ss)
    return m.astype(NPF16)


def kernel(a, x, Wq, Wk, Wv, Wo, cos, sin):
    a = np.asarray(a, np.float32)
    x = np.asarray(x, np.float32)
    Wq_p, Wk_p, Wv_h, Wo_h, cosT, sinT = _host_prep(
        np.asarray(Wq, np.float32), np.asarray(Wk, np.float32),
        np.asarray(Wv, np.float32), np.asarray(Wo, np.float32),
        np.asarray(cos, np.float32), np.asarray(sin, np.float32))

    # ---- phase A: k + v ----
    nca = build_phase_a()
    in_a = []
    for core in range(N_CORES):
        b, s4 = divmod(core, 4)
        rows = slice(512 * s4, 512 * (s4 + 1))
        in_a.append({
            "xT": np.ascontiguousarray(x[b].T[:, rows]).astype(NPF16),
            "aTv": np.ascontiguousarray(a[b].T[:, rows]).astype(NPF16),
            "Wk": Wk_p, "Wv": Wv_h,
            "cosA": np.ascontiguousarray(cosT[:, rows]),
            "sinA": np.ascontiguousarray(sinT[:, rows]),
        })
    res_a = run_bass_kernel_spmd(nca, in_a, list(range(N_CORES)))

    kr_full = [np.concatenate([res_a.results[b * 4 + s]["krA"] for s in range(4)], axis=1)
               for b in range(B)]   # [C, T] fp16
    v_full = [np.concatenate([res_a.results[b * 4 + s]["vA"] for s in range(4)], axis=0)
              for b in range(B)]    # [T, NB*C] fp16

    # ---- phase B: q + attention ----
    ncb = build_phase_b()
    in_b = []
    for core in range(N_CORES):
        b, j = divmod(core, 4)
        tcols = np.concatenate([np.arange(BLK * tb, BLK * (tb + 1))
                                for tb in _core_blocks(j)])
        in_b.append({
            "aQ": np.ascontiguousarray(a[b].T[:, tcols]).astype(NPF16),
            "Wq": Wq_p,
            "cosB": np.ascontiguousarray(cosT[:, tcols]),
            "sinB": np.ascontiguousarray(sinT[:, tcols]),
            "krB": kr_full[b],
            "vB": v_full[b],
            "Wo": Wo_h,
            "mskd": _masks(j),
        })
    res_b = run_bass_kernel_spmd(ncb, in_b, list(range(N_CORES)))

    outf = np.zeros((B, T, C), np.float32)
    for core in range(N_CORES):
        b, j = divmod(core, 4)
        o = res_b.results[core]["o"]
        for sl, tb in enumerate(_core_blocks(j)):
            outf[b, BLK * tb:BLK * (tb + 1)] = o[sl * 128:(sl + 1) * 128]
    return outf
